# revision 10
# baseline (speedup 1.0000x reference)
"""Multi-head attention (B=2, N=2048, C=1024, H=16, D=64) on 8 TRN2 cores.

Sharding: tensor-parallel over heads — 2 heads per core. Each core computes
Q/K/V projections for its 2 heads, attention, and a partial output
projection (its heads' slice of Wo). Host sums the 8 partial outputs + bo.

Per-core dataflow (all matmul inputs bf16, PSUM accumulation fp32):
  xT [1024, 4096] (x transposed on host, replicated to all cores)
  QT/KT = W.T @ x.T   -> [128 (2 heads x 64), 4096]  (lhsT=W chunk, rhs=xT)
  VT likewise, then PE-transposed into v_aug [keys, 65] per head
  (65th column = ones -> softmax denominator comes out of the ctx matmul)
  S^T = K @ Q.T  -> [keys, q] in PSUM; exp on ScalarE -> bf16 SBUF
  ctx^T_aug [65, q] = v_aug.T @ expS^T  (row 64 = denominator)
  normalize: recip(row 64), gpsimd partition_broadcast, DVE multiply
  out_partial [4096, 1024] = ctx^T.T @ Wo_slice  (fp32 out, summed on host)

The 1/sqrt(D) scale is folded into Wq/bq on the host (exact: 0.125).
"""

import numpy as np
import ml_dtypes

import concourse.bass as bass
from concourse import bacc
import concourse.tile as tile
from concourse import mybir, library_config
from concourse.bass_utils import run_bass_kernel_spmd

BF16 = mybir.dt.bfloat16
F32 = mybir.dt.float32

B, N, C = 2, 2048, 1024
H, D = 16, 64
T = B * N              # 4096 tokens
HPC = H // 8           # heads per core = 2
DPC = HPC * D          # head dims per core = 128


def build_core_program(nc):
    """Emit the per-core SPMD program. Same program on all 8 cores;
    per-core data differences come from the input maps."""
    xT = nc.dram_tensor("xT", [C, T], BF16, kind="ExternalInput").ap()
    wq = nc.dram_tensor("wq", [C, DPC], BF16, kind="ExternalInput").ap()
    wk = nc.dram_tensor("wk", [C, DPC], BF16, kind="ExternalInput").ap()
    wv = nc.dram_tensor("wv", [C, DPC], BF16, kind="ExternalInput").ap()
    wo = nc.dram_tensor("wo", [DPC, C], BF16, kind="ExternalInput").ap()
    bq = nc.dram_tensor("bq", [DPC, 1], F32, kind="ExternalInput").ap()
    bk = nc.dram_tensor("bk", [DPC, 1], F32, kind="ExternalInput").ap()
    bv = nc.dram_tensor("bv", [DPC, 1], F32, kind="ExternalInput").ap()
    iden = nc.dram_tensor("iden", [128, 128], BF16, kind="ExternalInput").ap()
    out = nc.dram_tensor("out", [T, C], F32, kind="ExternalOutput").ap()

    KCH = C // 128     # 8 contraction chunks for projections
    NCH = T // 512     # 8 token chunks of 512
    KT16 = N // 128    # 16 key tiles per batch

    with tile.TileContext(nc) as tc:
        with tc.tile_pool(name="singles", bufs=1) as singles:
            nc.gpsimd.load_library(library_config.attn)

            id_sb = singles.tile([128, 128], BF16, tag="iden")
            nc.sync.dma_start(out=id_sb, in_=iden)

            w_sb = {}
            for nm, w in (("wq", wq), ("wk", wk), ("wv", wv)):
                w_sb[nm] = []
                for k in range(KCH):
                    t = singles.tile([128, DPC], BF16, tag=f"{nm}{k}")
                    nc.sync.dma_start(out=t, in_=w[k * 128:(k + 1) * 128, :])
                    w_sb[nm].append(t)
            wo_sb = singles.tile([DPC, C], BF16, tag="wo")
            nc.sync.dma_start(out=wo_sb, in_=wo)

            b_sb = {}
            for nm, bsrc in (("bq", bq), ("bk", bk), ("bv", bv)):
                t = singles.tile([DPC, 1], F32, tag=f"b{nm}")
                nc.sync.dma_start(out=t, in_=bsrc)
                b_sb[nm] = t

            # xT resident in SBUF as 8x4 tiles [128, 1024] so the first
            # projection matmuls start after ~256KB of DMA, not 8MB.
            xt = [[singles.tile([128, 1024], BF16, tag=f"xt{k}_{c}", name=f"xt{k}_{c}")
                   for c in range(4)] for k in range(KCH)]
            for c in range(4):
                for k in range(KCH):
                    nc.sync.dma_start(
                        out=xt[k][c],
                        in_=xT[k * 128:(k + 1) * 128, c * 1024:(c + 1) * 1024])

            QT = singles.tile([128, T], BF16, tag="QT")
            KTt = singles.tile([128, T], BF16, tag="KT")
            VT = singles.tile([128, T], BF16, tag="VT")
            ctxTn = singles.tile([128, T], BF16, tag="ctxTn")
            vaug = [[singles.tile([128, KT16, D + 1], BF16, tag=f"vaug{b}{h}", name=f"vaug{b}{h}")
                     for h in range(HPC)] for b in range(B)]
            for b in range(B):
                for h in range(HPC):
                    nc.vector.memset(vaug[b][h], 1.0)

            # One unified PSUM layout for the whole kernel so projections and
            # attention can overlap freely (8 banks: pj 2 + s 4 + ctx 2).
            # Emission order interleaves per-batch: V+transposes and K for a
            # batch, then per q-chunk the matching Q projection followed by
            # that chunk's attention — later projections fill PE bubbles
            # while ACT works through the exps.
            with tc.tile_pool(name="psP", bufs=2, space="PSUM") as psP, \
                    tc.tile_pool(name="psS", bufs=2, space="PSUM") as psS, \
                    tc.tile_pool(name="psC", bufs=1, space="PSUM") as psC, \
                    tc.tile_pool(name="esb", bufs=3) as esb, \
                    tc.tile_pool(name="nrm", bufs=2) as nrm, \
                    tc.tile_pool(name="csb", bufs=2) as csb, \
                    tc.tile_pool(name="osb", bufs=3) as osb:

                def emit_proj(nm, dstT, nch):
                    ps = psP.tile([128, 512], F32, tag="pj", name="pj")
                    c, off = divmod(nch * 512, 1024)
                    for k in range(KCH):
                        nc.tensor.matmul(
                            out=ps, lhsT=w_sb[nm][k],
                            rhs=xt[k][c][:, off:off + 512],
                            start=(k == 0), stop=(k == KCH - 1))
                    nc.vector.tensor_scalar_add(
                        out=dstT[:, nch * 512:(nch + 1) * 512],
                        in0=ps, scalar1=b_sb["b" + nm[1]])
                    if nm == "wv":
                        # transpose the 4 just-projected 128-token tiles of V
                        # into v_aug [keys, 65] per head
                        for t16 in range(nch * 4, nch * 4 + 4):
                            b, bt = divmod(t16, KT16)
                            pt = psP.tile([128, 128], BF16, tag="pj",
                                          name="pt")
                            base = t16 * 128
                            nc.tensor.transpose(
                                pt, VT[:, base:base + 128], id_sb)
                            nc.vector.tensor_copy(
                                out=vaug[b][0][:, bt, 0:D], in_=pt[:, 0:D])
                            nc.vector.tensor_copy(
                                out=vaug[b][1][:, bt, 0:D], in_=pt[:, D:2 * D])

                def emit_attention(b, qch):
                    q0 = b * N + qch * 512
                    ctx = [psC.tile([D + 1, 512], F32, tag=f"ctx{h}",
                                    name=f"ctx{h}") for h in range(HPC)]
                    for kc in range(KT16):
                        k0 = b * N + kc * 128
                        pS = psS.tile([128, 1024], F32, tag="s", name="s")
                        for h in range(HPC):
                            nc.tensor.matmul(
                                out=pS[:, h * 512:(h + 1) * 512],
                                lhsT=KTt[h * D:(h + 1) * D, k0:k0 + 128],
                                rhs=QT[h * D:(h + 1) * D, q0:q0 + 512],
                                start=True, stop=True)
                        eS = esb.tile([128, 1024], BF16, tag="e", name="e")
                        nc.scalar.activation(
                            eS, pS, mybir.ActivationFunctionType.Exp)
                        for h in range(HPC):
                            nc.tensor.matmul(
                                out=ctx[h],
                                lhsT=vaug[b][h][:, kc, :],
                                rhs=eS[:, h * 512:(h + 1) * 512],
                                start=(kc == 0), stop=(kc == KT16 - 1))
                    # normalize: rows 0..63 / row 64, into stacked ctxTn.
                    # Copy PSUM->SBUF first so the accumulator bank frees
                    # immediately and the recip/bcast/mul chain runs off the
                    # PE critical path.
                    for h in range(HPC):
                        dn = nrm.tile([1, 512], F32, tag=f"dn{h}",
                                      name=f"dn{h}")
                        nc.vector.tensor_copy(dn, ctx[h][D:D + 1, :])
                        ctxs = csb.tile([D, 512], F32, tag=f"ctxs{h}",
                                        name=f"ctxs{h}")
                        nc.vector.tensor_copy(ctxs, ctx[h][0:D, :])
                        rc = nrm.tile([1, 512], F32, tag=f"rc{h}",
                                      name=f"rc{h}")
                        nc.vector.reciprocal_approx_fast(rc, dn)
                        bc = nrm.tile([D, 512], F32, tag=f"bc{h}",
                                      name=f"bc{h}")
                        nc.gpsimd.partition_broadcast(bc, rc)
                        nc.vector.tensor_mul(
                            out=ctxTn[h * D:(h + 1) * D, q0:q0 + 512],
                            in0=ctxs, in1=bc)
                    # output projection for this q chunk
                    for t4 in range(4):
                        tok = q0 + t4 * 128
                        for nch2 in range(2):
                            po = psP.tile([128, 512], F32, tag="pj",
                                          name="po")
                            nc.tensor.matmul(
                                out=po,
                                lhsT=ctxTn[:, tok:tok + 128],
                                rhs=wo_sb[:, nch2 * 512:(nch2 + 1) * 512],
                                start=True, stop=True)
                            ot = osb.tile([128, 512], F32, tag="ot",
                                          name="ot")
                            nc.vector.tensor_copy(ot, po)
                            nc.sync.dma_start(
                                out=out[tok:tok + 128,
                                        nch2 * 512:(nch2 + 1) * 512],
                                in_=ot)

                for b in range(B):
                    half = b * 4
                    for nch in range(half, half + 4):
                        emit_proj("wv", VT, nch)
                    for nch in range(half, half + 4):
                        emit_proj("wk", KTt, nch)
                    for qch in range(4):
                        emit_proj("wq", QT, half + qch)
                        emit_attention(b, qch)
    return nc


_NC_CACHE = None


def _get_nc():
    global _NC_CACHE
    if _NC_CACHE is None:
        nc = bacc.Bacc("TRN2", target_bir_lowering=False)
        build_core_program(nc)
        nc.finalize()
        _NC_CACHE = nc
    return _NC_CACHE


def make_in_maps(x, Wq, bq, Wk, bk, Wv, bv, Wo):
    bf = ml_dtypes.bfloat16
    x = np.asarray(x, np.float32).reshape(T, C)
    xT_bf = np.ascontiguousarray(x.T).astype(bf)
    iden = np.eye(128, dtype=bf)
    Wq = np.asarray(Wq, np.float32)
    Wk = np.asarray(Wk, np.float32)
    Wv = np.asarray(Wv, np.float32)
    Wo = np.asarray(Wo, np.float32)
    bq = np.asarray(bq, np.float32)
    bk = np.asarray(bk, np.float32)
    bv = np.asarray(bv, np.float32)
    in_maps = []
    for cidx in range(8):
        hs = slice(cidx * DPC, (cidx + 1) * DPC)
        in_maps.append(dict(
            xT=xT_bf,
            wq=np.ascontiguousarray(Wq[:, hs] * 0.125).astype(bf),
            wk=np.ascontiguousarray(Wk[:, hs]).astype(bf),
            wv=np.ascontiguousarray(Wv[:, hs]).astype(bf),
            wo=np.ascontiguousarray(Wo[hs, :]).astype(bf),
            bq=(bq[hs] * 0.125).astype(np.float32).reshape(DPC, 1),
            bk=bk[hs].astype(np.float32).reshape(DPC, 1),
            bv=bv[hs].astype(np.float32).reshape(DPC, 1),
            iden=iden,
        ))
    return in_maps


def kernel(x, Wq, bq, Wk, bk, Wv, bv, Wo, bo, _trace=False, _trace_kwargs=None):
    in_maps = make_in_maps(x, Wq, bq, Wk, bk, Wv, bv, Wo)
    nc = _get_nc()
    res = run_bass_kernel_spmd(
        nc, in_maps, core_ids=list(range(8)),
        trace=_trace, **(_trace_kwargs or {}))
    acc = res.results[0]["out"].copy()
    for cidx in range(1, 8):
        acc += res.results[cidx]["out"]
    acc += np.asarray(bo, np.float32)[None, :]
    out = acc.reshape(B, N, C)
    kernel.last_results = res
    return out


# revision 11
# speedup vs baseline: 1.5247x; 1.5247x over previous
"""Multi-head attention (B=2, N=2048, C=1024, H=16, D=64) on 8 TRN2 cores.

Sharding: tensor-parallel over heads — 2 heads per core. Each core computes
Q/K/V projections for its 2 heads, attention, and a partial output
projection (its heads' slice of Wo). Host sums the 8 partial outputs + bo.

Per-core dataflow (all matmul inputs bf16, PSUM accumulation fp32):
  xT [1024, 4096] (x transposed on host, replicated to all cores)
  QT/KT = W.T @ x.T   -> [128 (2 heads x 64), 4096]  (lhsT=W chunk, rhs=xT)
  VT likewise, then PE-transposed into v_aug [keys, 65] per head
  (65th column = ones -> softmax denominator comes out of the ctx matmul)
  S^T = K @ Q.T  -> [keys, q] in PSUM; exp on ScalarE -> bf16 SBUF
  ctx^T_aug [65, q] = v_aug.T @ expS^T  (row 64 = denominator)
  normalize: recip(row 64), gpsimd partition_broadcast, DVE multiply
  out_partial [4096, 1024] = ctx^T.T @ Wo_slice  (fp32 out, summed on host)

The 1/sqrt(D) scale is folded into Wq/bq on the host (exact: 0.125).
"""

import numpy as np
import ml_dtypes

import concourse.bass as bass
from concourse import bacc
import concourse.tile as tile
from concourse import mybir, library_config
from concourse.bass_utils import run_bass_kernel_spmd

BF16 = mybir.dt.bfloat16
F32 = mybir.dt.float32

B, N, C = 2, 2048, 1024
H, D = 16, 64
T = B * N              # 4096 tokens
HPC = H // 8           # heads per core = 2
DPC = HPC * D          # head dims per core = 128


def build_core_program(nc):
    """Emit the per-core SPMD program. Same program on all 8 cores;
    per-core data differences come from the input maps."""
    xT = nc.dram_tensor("xT", [C, T], BF16, kind="ExternalInput").ap()
    wq = nc.dram_tensor("wq", [C, DPC], BF16, kind="ExternalInput").ap()
    wk = nc.dram_tensor("wk", [C, DPC], BF16, kind="ExternalInput").ap()
    wv = nc.dram_tensor("wv", [C, DPC], BF16, kind="ExternalInput").ap()
    wo = nc.dram_tensor("wo", [DPC, C], BF16, kind="ExternalInput").ap()
    bq = nc.dram_tensor("bq", [DPC, 1], F32, kind="ExternalInput").ap()
    bk = nc.dram_tensor("bk", [DPC, 1], F32, kind="ExternalInput").ap()
    bv = nc.dram_tensor("bv", [DPC, 1], F32, kind="ExternalInput").ap()
    iden = nc.dram_tensor("iden", [128, 128], BF16, kind="ExternalInput").ap()
    out = nc.dram_tensor("out", [T, C], F32, kind="ExternalOutput").ap()

    KCH = C // 128     # 8 contraction chunks for projections
    NCH = T // 512     # 8 token chunks of 512
    KT16 = N // 128    # 16 key tiles per batch

    with tile.TileContext(nc) as tc:
        with tc.tile_pool(name="singles", bufs=1) as singles:
            nc.gpsimd.load_library(library_config.attn)

            id_sb = singles.tile([128, 128], BF16, tag="iden")
            nc.sync.dma_start(out=id_sb, in_=iden)

            w_sb = {}
            for nm, w in (("wq", wq), ("wk", wk), ("wv", wv)):
                w_sb[nm] = []
                for k in range(KCH):
                    t = singles.tile([128, DPC], BF16, tag=f"{nm}{k}")
                    nc.sync.dma_start(out=t, in_=w[k * 128:(k + 1) * 128, :])
                    w_sb[nm].append(t)
            wo_sb = singles.tile([DPC, C], BF16, tag="wo")
            nc.sync.dma_start(out=wo_sb, in_=wo)

            b_sb = {}
            for nm, bsrc in (("bq", bq), ("bk", bk), ("bv", bv)):
                t = singles.tile([DPC, 1], F32, tag=f"b{nm}")
                nc.sync.dma_start(out=t, in_=bsrc)
                b_sb[nm] = t

            # xT resident in SBUF as 8x4 tiles [128, 1024] so the first
            # projection matmuls start after ~256KB of DMA, not 8MB.
            xt = [[singles.tile([128, 1024], BF16, tag=f"xt{k}_{c}", name=f"xt{k}_{c}")
                   for c in range(4)] for k in range(KCH)]
            for c in range(4):
                for k in range(KCH):
                    nc.sync.dma_start(
                        out=xt[k][c],
                        in_=xT[k * 128:(k + 1) * 128, c * 1024:(c + 1) * 1024])

            QT = singles.tile([128, T], BF16, tag="QT")
            KTt = singles.tile([128, T], BF16, tag="KT")
            VT = singles.tile([128, T], BF16, tag="VT")
            ctxTn = singles.tile([128, T], BF16, tag="ctxTn")
            vaug = [[singles.tile([128, KT16, D + 1], BF16, tag=f"vaug{b}{h}", name=f"vaug{b}{h}")
                     for h in range(HPC)] for b in range(B)]
            for b in range(B):
                for h in range(HPC):
                    nc.vector.memset(vaug[b][h], 1.0)

            # One unified PSUM layout for the whole kernel so projections and
            # attention can overlap freely (8 banks: pj 2 + s 4 + ctx 2).
            # Emission order interleaves per-batch: V+transposes and K for a
            # batch, then per q-chunk the matching Q projection followed by
            # that chunk's attention — later projections fill PE bubbles
            # while ACT works through the exps.
            with tc.tile_pool(name="psP", bufs=1, space="PSUM") as psP, \
                    tc.tile_pool(name="psO", bufs=1, space="PSUM") as psO, \
                    tc.tile_pool(name="psS", bufs=2, space="PSUM") as psS, \
                    tc.tile_pool(name="psC", bufs=1, space="PSUM") as psC, \
                    tc.tile_pool(name="esb", bufs=3) as esb, \
                    tc.tile_pool(name="nrm", bufs=2) as nrm, \
                    tc.tile_pool(name="csb", bufs=2) as csb, \
                    tc.tile_pool(name="osb", bufs=3) as osb:

                def emit_proj(nm, dstT, nch):
                    ps = psP.tile([128, 512], F32, tag="pj", name="pj")
                    c, off = divmod(nch * 512, 1024)
                    for k in range(KCH):
                        nc.tensor.matmul(
                            out=ps, lhsT=w_sb[nm][k],
                            rhs=xt[k][c][:, off:off + 512],
                            start=(k == 0), stop=(k == KCH - 1))
                    nc.vector.tensor_scalar_add(
                        out=dstT[:, nch * 512:(nch + 1) * 512],
                        in0=ps, scalar1=b_sb["b" + nm[1]])
                    if nm == "wv":
                        # transpose the 4 just-projected 128-token tiles of V
                        # into v_aug [keys, 65] per head
                        for t16 in range(nch * 4, nch * 4 + 4):
                            b, bt = divmod(t16, KT16)
                            pt = psO.tile([128, 128], BF16, tag="po",
                                          name="pt")
                            base = t16 * 128
                            nc.tensor.transpose(
                                pt, VT[:, base:base + 128], id_sb)
                            nc.vector.tensor_copy(
                                out=vaug[b][0][:, bt, 0:D], in_=pt[:, 0:D])
                            nc.vector.tensor_copy(
                                out=vaug[b][1][:, bt, 0:D], in_=pt[:, D:2 * D])

                def emit_attention(b, qch):
                    q0 = b * N + qch * 512
                    ctx = [psC.tile([D + 1, 512], F32, tag=f"ctx{h}",
                                    name=f"ctx{h}") for h in range(HPC)]
                    for kc in range(KT16):
                        k0 = b * N + kc * 128
                        pS = psS.tile([128, 1024], F32, tag="s", name="s")
                        for h in range(HPC):
                            nc.tensor.matmul(
                                out=pS[:, h * 512:(h + 1) * 512],
                                lhsT=KTt[h * D:(h + 1) * D, k0:k0 + 128],
                                rhs=QT[h * D:(h + 1) * D, q0:q0 + 512],
                                start=True, stop=True)
                        eS = esb.tile([128, 1024], BF16, tag="e", name="e")
                        nc.scalar.activation(
                            eS, pS, mybir.ActivationFunctionType.Exp)
                        for h in range(HPC):
                            nc.tensor.matmul(
                                out=ctx[h],
                                lhsT=vaug[b][h][:, kc, :],
                                rhs=eS[:, h * 512:(h + 1) * 512],
                                start=(kc == 0), stop=(kc == KT16 - 1))
                    # normalize: rows 0..63 / row 64, into stacked ctxTn.
                    # Copy PSUM->SBUF first so the accumulator bank frees
                    # immediately and the recip/bcast/mul chain runs off the
                    # PE critical path.
                    for h in range(HPC):
                        dn = nrm.tile([1, 512], F32, tag=f"dn{h}",
                                      name=f"dn{h}")
                        nc.vector.tensor_copy(dn, ctx[h][D:D + 1, :])
                        ctxs = csb.tile([D, 512], F32, tag=f"ctxs{h}",
                                        name=f"ctxs{h}")
                        nc.vector.tensor_copy(ctxs, ctx[h][0:D, :])
                        rc = nrm.tile([1, 512], F32, tag=f"rc{h}",
                                      name=f"rc{h}")
                        nc.vector.reciprocal_approx_fast(rc, dn)
                        bc = nrm.tile([D, 512], F32, tag=f"bc{h}",
                                      name=f"bc{h}")
                        nc.gpsimd.partition_broadcast(bc, rc)
                        nc.vector.tensor_mul(
                            out=ctxTn[h * D:(h + 1) * D, q0:q0 + 512],
                            in0=ctxs, in1=bc)
                    # output projection for this q chunk
                    for t4 in range(4):
                        tok = q0 + t4 * 128
                        for nch2 in range(2):
                            po = psO.tile([128, 512], F32, tag="po",
                                          name="po")
                            nc.tensor.matmul(
                                out=po,
                                lhsT=ctxTn[:, tok:tok + 128],
                                rhs=wo_sb[:, nch2 * 512:(nch2 + 1) * 512],
                                start=True, stop=True)
                            ot = osb.tile([128, 512], F32, tag="ot",
                                          name="ot")
                            nc.vector.tensor_copy(ot, po)
                            nc.sync.dma_start(
                                out=out[tok:tok + 128,
                                        nch2 * 512:(nch2 + 1) * 512],
                                in_=ot)

                for b in range(B):
                    half = b * 4
                    for nch in range(half, half + 4):
                        emit_proj("wv", VT, nch)
                    for nch in range(half, half + 4):
                        emit_proj("wk", KTt, nch)
                    for qch in range(4):
                        emit_proj("wq", QT, half + qch)
                        emit_attention(b, qch)
    return nc


_NC_CACHE = None


def _get_nc():
    global _NC_CACHE
    if _NC_CACHE is None:
        nc = bacc.Bacc("TRN2", target_bir_lowering=False)
        build_core_program(nc)
        nc.finalize()
        _NC_CACHE = nc
    return _NC_CACHE


def make_in_maps(x, Wq, bq, Wk, bk, Wv, bv, Wo):
    bf = ml_dtypes.bfloat16
    x = np.asarray(x, np.float32).reshape(T, C)
    xT_bf = np.ascontiguousarray(x.T).astype(bf)
    iden = np.eye(128, dtype=bf)
    Wq = np.asarray(Wq, np.float32)
    Wk = np.asarray(Wk, np.float32)
    Wv = np.asarray(Wv, np.float32)
    Wo = np.asarray(Wo, np.float32)
    bq = np.asarray(bq, np.float32)
    bk = np.asarray(bk, np.float32)
    bv = np.asarray(bv, np.float32)
    in_maps = []
    for cidx in range(8):
        hs = slice(cidx * DPC, (cidx + 1) * DPC)
        in_maps.append(dict(
            xT=xT_bf,
            wq=np.ascontiguousarray(Wq[:, hs] * 0.125).astype(bf),
            wk=np.ascontiguousarray(Wk[:, hs]).astype(bf),
            wv=np.ascontiguousarray(Wv[:, hs]).astype(bf),
            wo=np.ascontiguousarray(Wo[hs, :]).astype(bf),
            bq=(bq[hs] * 0.125).astype(np.float32).reshape(DPC, 1),
            bk=bk[hs].astype(np.float32).reshape(DPC, 1),
            bv=bv[hs].astype(np.float32).reshape(DPC, 1),
            iden=iden,
        ))
    return in_maps


def kernel(x, Wq, bq, Wk, bk, Wv, bv, Wo, bo, _trace=False, _trace_kwargs=None):
    in_maps = make_in_maps(x, Wq, bq, Wk, bk, Wv, bv, Wo)
    nc = _get_nc()
    res = run_bass_kernel_spmd(
        nc, in_maps, core_ids=list(range(8)),
        trace=_trace, **(_trace_kwargs or {}))
    acc = res.results[0]["out"].copy()
    for cidx in range(1, 8):
        acc += res.results[cidx]["out"]
    acc += np.asarray(bo, np.float32)[None, :]
    out = acc.reshape(B, N, C)
    kernel.last_results = res
    return out
